# revision 1
# baseline (speedup 1.0000x reference)
"""Trainium2 Bass kernel for nn_MultiHeadAttention (B=4, S=2048, d_model=1024, 16 heads).

Sharding: Megatron-style head-parallel across 8 NeuronCores (2 heads / core).
Each core computes q/k/v projections for its 128-column slice of the head dim,
full attention for its 2 heads over all 4 batches, and a partial (row-slice)
out-projection. Host sums the 8 partials and adds the output bias.

Layout strategy: activations are transposed on host to [d_model, B*S] so the
PE (which contracts over the partition dim) consumes them directly. q/k are
produced transposed ([j, s]); v is produced natural ([s, j]) and packed with a
ones column so attn@v yields both the unnormalized output and the softmax
denominators in one accumulation chain. The out-projection emits a transposed
[1024, 8192] partial per core; the host un-transposes once after summing.
All matmuls run in bf16 with fp32 PSUM accumulation.
"""
import sys

sys.path.insert(0, "/opt/trn_rl_repo")

import numpy as np
import ml_dtypes

import concourse.bacc as bacc
import concourse.tile as tile
from concourse import mybir

B, S, D, H, DK = 4, 2048, 1024, 16, 64
NCORES = 8
JC = (H // NCORES) * DK  # 128 head-columns per core
BS = B * S  # 8192
SQC = 512  # q-window / projection free-dim chunk
NSQ = S // SQC  # 4 q-windows per batch
NSK = S // 128  # 16 key chunks per batch
NIC = D // 128  # 8 contraction chunks for projections
NSC = BS // SQC  # 16 s-chunks for projections
NICOUT = D // 128  # 8 output i-chunks

BF16 = mybir.dt.bfloat16
F32 = mybir.dt.float32
AF = mybir.ActivationFunctionType
bf16 = ml_dtypes.bfloat16

_CACHE = {}


def _build_bass(niter=1):
    from contextlib import nullcontext

    nc = bacc.Bacc("TRN2", target_bir_lowering=False, debug=False)
    xq = nc.dram_tensor("xq", [NSC, 128, NIC, SQC], BF16, kind="ExternalInput")
    xk = nc.dram_tensor("xk", [NSC, 128, NIC, SQC], BF16, kind="ExternalInput")
    xv = nc.dram_tensor("xv", [NSC, 128, NIC, SQC], BF16, kind="ExternalInput")
    wq = nc.dram_tensor("wq", [128, NIC, JC], BF16, kind="ExternalInput")
    wk = nc.dram_tensor("wk", [128, NIC, JC], BF16, kind="ExternalInput")
    wv = nc.dram_tensor("wv", [128, NIC, JC], BF16, kind="ExternalInput")
    wo = nc.dram_tensor("wo", [JC, D], BF16, kind="ExternalInput")
    bq = nc.dram_tensor("bq", [JC, 1], F32, kind="ExternalInput")
    bk = nc.dram_tensor("bk", [JC, 1], F32, kind="ExternalInput")
    bv = nc.dram_tensor("bv", [1, JC], F32, kind="ExternalInput")
    outT = nc.dram_tensor("outT", [D, BS], BF16, kind="ExternalOutput")

    with tile.TileContext(nc) as tc:
        with (
            tc.tile_pool(name="consts", bufs=1) as consts,
            tc.tile_pool(name="xin", bufs=4) as xin,
            tc.tile_pool(name="big", bufs=1) as big,
            tc.tile_pool(name="work", bufs=6) as work,
            tc.tile_pool(name="ps", bufs=2, space="PSUM") as ps,
            tc.For_i(0, niter, 1) if niter > 1 else nullcontext(),
        ):
            wq_sb = consts.tile([128, NIC, JC], BF16)
            wk_sb = consts.tile([128, NIC, JC], BF16)
            wv_sb = consts.tile([128, NIC, JC], BF16)
            wo_sb = consts.tile([JC, D], BF16)
            bq_sb = consts.tile([JC, 1], F32)
            bk_sb = consts.tile([JC, 1], F32)
            bvb_sb = consts.tile([128, JC], F32)
            nc.sync.dma_start(wq_sb[:], wq[:])
            nc.sync.dma_start(wk_sb[:], wk[:])
            nc.sync.dma_start(wv_sb[:], wv[:])
            nc.sync.dma_start(wo_sb[:], wo[:])
            nc.sync.dma_start(bq_sb[:], bq[:])
            nc.sync.dma_start(bk_sb[:], bk[:])
            # broadcast the v bias across all 128 partitions (free-step-0 source)
            nc.sync.dma_start(bvb_sb[:], bv[:, None, :].broadcast_to([1, 128, JC]))

            qT_sb = big.tile([128, BS], BF16)
            kT_sb = big.tile([128, BS], BF16)
            aoT_sb = big.tile([128, BS], BF16)
            v1_sb = big.tile([128, B, 2, NSK, 65], BF16)
            nc.vector.memset(v1_sb[:], 1.0)  # ones column at [..., 64]; rest overwritten

            # ---- q/k projections -> transposed [j, s] with fused bias ----
            for xdram, w_sb, b_sb, dest in (
                (xq, wq_sb, bq_sb, qT_sb),
                (xk, wk_sb, bk_sb, kT_sb),
            ):
                for sc in range(NSC):
                    xt = xin.tile([128, NIC, SQC], BF16, tag="xt")
                    nc.sync.dma_start(xt[:], xdram[sc])
                    pq = ps.tile([128, SQC], F32, tag="mm", bufs=2)
                    for ic in range(NIC):
                        nc.tensor.matmul(
                            pq[:], w_sb[:, ic, :], xt[:, ic, :],
                            start=(ic == 0), stop=(ic == NIC - 1),
                        )
                    nc.vector.tensor_add(
                        dest[:, sc * SQC : (sc + 1) * SQC],
                        pq[:],
                        b_sb[:].broadcast_to([JC, SQC]),
                    )

            # ---- v projection -> natural [s, j], packed into v1 with bias ----
            for sc in range(NSC):
                xt = xin.tile([128, NIC, SQC], BF16, tag="xt")
                nc.sync.dma_start(xt[:], xv[sc])
                for sub in range(SQC // 128):
                    pv = ps.tile([128, JC], F32, tag="mm", bufs=2)
                    for ic in range(NIC):
                        nc.tensor.matmul(
                            pv[:], xt[:, ic, sub * 128 : (sub + 1) * 128], wv_sb[:, ic, :],
                            start=(ic == 0), stop=(ic == NIC - 1),
                        )
                    sg = sc * (SQC // 128) + sub
                    b_, skc = divmod(sg, NSK)
                    for h in range(2):
                        nc.vector.tensor_add(
                            v1_sb[:, b_, h, skc, 0:64],
                            pv[:, h * 64 : (h + 1) * 64],
                            bvb_sb[:, h * 64 : (h + 1) * 64],
                        )

            # ---- attention: scoresT -> exp -> [v|1]^T @ P^T -> normalize ----
            for b_ in range(B):
                for sqc in range(NSQ):
                    w = slice(b_ * S + sqc * SQC, b_ * S + (sqc + 1) * SQC)
                    oA = ps.tile([65, SQC], F32, tag="psO", bufs=2)
                    oB = ps.tile([65, SQC], F32, tag="psO", bufs=2)
                    for skc in range(NSK):
                        kk = slice(b_ * S + skc * 128, b_ * S + (skc + 1) * 128)
                        sA = ps.tile([128, SQC], F32, tag="psS", bufs=4)
                        sB = ps.tile([128, SQC], F32, tag="psS", bufs=4)
                        # two K=64 heads packed as concurrent PE row-tiles
                        nc.tensor.matmul(sA[:], kT_sb[0:64, kk], qT_sb[0:64, w], start=True, stop=True)
                        nc.tensor.matmul(sB[:], kT_sb[64:128, kk], qT_sb[64:128, w], start=True, stop=True)
                        ptA = work.tile([128, SQC], BF16, tag="pt", bufs=8)
                        ptB = work.tile([128, SQC], BF16, tag="pt", bufs=8)
                        nc.scalar.activation(ptA[:], sA[:], AF.Exp, scale=0.125)
                        nc.scalar.activation(ptB[:], sB[:], AF.Exp, scale=0.125)
                        nc.tensor.matmul(oA[:], v1_sb[:, b_, 0, skc, :], ptA[:], start=(skc == 0), stop=(skc == NSK - 1))
                        nc.tensor.matmul(oB[:], v1_sb[:, b_, 1, skc, :], ptB[:], start=(skc == 0), stop=(skc == NSK - 1))
                    rA = work.tile([1, SQC], F32, tag="recip", bufs=2)
                    rB = work.tile([1, SQC], F32, tag="recip", bufs=2)
                    nc.vector.reciprocal(rA[:], oA[64:65, :])
                    nc.vector.reciprocal(rB[:], oB[64:65, :])
                    bc = work.tile([128, SQC], F32, tag="bc", bufs=2)
                    nc.scalar.dma_start(bc[0:64, :], rA[:, None, :].broadcast_to([1, 64, SQC]))
                    nc.scalar.dma_start(bc[64:128, :], rB[:, None, :].broadcast_to([1, 64, SQC]))
                    nc.vector.tensor_mul(aoT_sb[0:64, w], oA[0:64, :], bc[0:64, :])
                    nc.vector.tensor_mul(aoT_sb[64:128, w], oB[0:64, :], bc[64:128, :])

            # ---- out projection (transposed): outT[i, s] = woT.T @ aoT ----
            for ic in range(NICOUT):
                for sc in range(NSC):
                    pf = ps.tile([128, SQC], F32, tag="mm", bufs=2)
                    nc.tensor.matmul(
                        pf[:], wo_sb[:, ic * 128 : (ic + 1) * 128],
                        aoT_sb[:, sc * SQC : (sc + 1) * SQC],
                        start=True, stop=True,
                    )
                    ft = work.tile([128, SQC], BF16, tag="ft", bufs=4)
                    nc.vector.tensor_copy(ft[:], pf[:])
                    nc.scalar.dma_start(
                        outT[ic * 128 : (ic + 1) * 128, sc * SQC : (sc + 1) * SQC], ft[:]
                    )
    nc.finalize()
    return nc


def _chunk_xT(x):
    """[B,S,D] f32 -> xT chunked [NSC, 128, NIC, SQC] bf16 (shared by all cores)."""
    xT = np.ascontiguousarray(x.reshape(BS, D).T.astype(bf16))  # [D, BS]
    return np.ascontiguousarray(
        xT.reshape(NIC, 128, NSC, SQC).transpose(2, 1, 0, 3)
    )


def _prep_inputs(query, key, value, Wq, bq, Wk, bk, Wv, bv, Wo):
    xq = _chunk_xT(query)
    xk = _chunk_xT(key)
    xv = _chunk_xT(value)
    in_maps = []
    for c in range(NCORES):
        sl = slice(c * JC, (c + 1) * JC)

        def wT(W):  # [1024,128] -> [128, NIC, JC] chunked lhsT layout
            t = np.ascontiguousarray(W[sl, :].T.astype(bf16))  # [D, JC]
            return np.ascontiguousarray(t.reshape(NIC, 128, JC).transpose(1, 0, 2))

        in_maps.append(
            {
                "xq": xq,
                "xk": xk,
                "xv": xv,
                "wq": wT(Wq),
                "wk": wT(Wk),
                "wv": wT(Wv),
                "wo": np.ascontiguousarray(Wo[:, sl].T.astype(bf16)),  # [JC, D]
                "bq": np.asarray(bq[sl], np.float32).reshape(JC, 1),
                "bk": np.asarray(bk[sl], np.float32).reshape(JC, 1),
                "bv": np.asarray(bv[sl], np.float32).reshape(1, JC),
            }
        )
    return in_maps


IN_NAMES = ["xq", "xk", "xv", "wq", "wk", "wv", "wo", "bq", "bk", "bv"]


def _get_mesh():
    import jax
    from jax.sharding import Mesh

    if "mesh" not in _CACHE:
        devices = jax.devices()[:NCORES]
        _CACHE["mesh"] = Mesh(np.asarray(devices), ("core",))
    return _CACHE["mesh"]


def _jitted_chain(niter):
    """Jitted runner for the Bass program with `niter` in-program iterations."""
    import jax
    from jax.sharding import PartitionSpec
    from jax.experimental.shard_map import shard_map
    from concourse import bass2jax

    key = ("jit", niter)
    if key in _CACHE:
        return _CACHE[key]

    nc = _CACHE.get(("nc", niter))
    if nc is None:
        nc = _CACHE[("nc", niter)] = _build_bass(niter)

    bass2jax.install_neuronx_cc_hook()
    out_avals = (jax.core.ShapedArray((D, BS), bf16),)
    part_name = nc.partition_id_tensor.name if nc.partition_id_tensor else None

    def _body(*args):
        operands = list(args)
        names = tuple(IN_NAMES)
        if part_name is not None:
            operands.append(bass2jax.partition_id_tensor())
            names = names + (part_name,)
        outs = bass2jax._bass_exec_p.bind(
            *operands,
            out_avals=out_avals,
            in_names=names,
            out_names=("outT",),
            lowering_input_output_aliases=(),
            sim_require_finite=True,
            sim_require_nnan=True,
            nc=nc,
        )
        return outs[0]

    fn = jax.jit(
        shard_map(
            _body,
            mesh=_get_mesh(),
            in_specs=(PartitionSpec("core"),) * len(IN_NAMES),
            out_specs=PartitionSpec("core"),
            check_rep=False,
        ),
        keep_unused=True,
    )
    _CACHE[key] = fn
    return fn


def _concat_inputs(in_maps):
    return [np.concatenate([m[name] for m in in_maps], axis=0) for name in IN_NAMES]


def _device_inputs(in_maps):
    """Stage per-core inputs onto the 8 devices once; reusable across calls."""
    import jax
    from jax.sharding import NamedSharding, PartitionSpec

    sh = NamedSharding(_get_mesh(), PartitionSpec("core"))
    return [jax.device_put(a, sh) for a in _concat_inputs(in_maps)]


def _timed_chain(in_maps, niter):
    """Wall-time one dispatch of the niter-iteration Bass program on
    device-resident inputs (the loop runs on-device; RPC cost is constant)."""
    import time

    dev = _CACHE.get("dev_inputs")
    if dev is None:
        dev = _CACHE["dev_inputs"] = _device_inputs(in_maps)
    fn = _jitted_chain(niter)
    fn(*dev).block_until_ready()  # compile+warm
    t0 = time.perf_counter()
    fn(*dev).block_until_ready()
    return time.perf_counter() - t0


def kernel(query, key, value, Wq, bq, Wk, bk, Wv, bv, Wo, bo):
    in_maps = _prep_inputs(query, key, value, Wq, bq, Wk, bk, Wv, bv, Wo)
    fn = _jitted_chain(1)
    out = np.asarray(fn(*_concat_inputs(in_maps)))  # [8*D, BS]
    acc = out[0:D].astype(np.float32)
    for c in range(1, NCORES):
        acc += out[c * D : (c + 1) * D]
    res = acc.T.reshape(B, S, D) + np.asarray(bo, np.float32)
    return np.ascontiguousarray(res.astype(np.float32))



# revision 21
# speedup vs baseline: 1.2145x; 1.2145x over previous
"""Trainium2 Bass kernel for nn_MultiHeadAttention (B=4, S=2048, d_model=1024, 16 heads).

Sharding: Megatron-style head-parallel across 8 NeuronCores (2 heads / core).
Each core computes q/k/v projections for its 128-column slice of the head dim,
full attention for its 2 heads over all 4 batches, and a partial (row-slice)
out-projection. Host sums the 8 partials and adds the output bias.

v2 design (engine-balanced, pipelined across batches):
- Projections: x is host-transposed to [d_model, B*S]; q/k produced transposed
  [j, s] via PE with a fused bias epilogue (tensor_scalar on Pool); v produced
  natural [s, j] packed with a ones column (softmax denominator trick).
- Attention per (batch, 512-wide q window, head): scoresT = k.T @ q into a
  2-bank PSUM pair, ONE 1024-wide exp on the Act engine (the bottleneck
  engine: exp is the only thing it runs), then attn@v in natural orientation
  (pt chunks as stationary, [v|1] as 65-wide moving) which runs at half the
  PE cost of the transposed form. Outputs are normalized via per-partition
  reciprocal scalars (no broadcast DMAs), transposed back with PE-transpose
  instructions, and assembled into aoT for the out-projection.
- Projections of batch b+1 and the out-projection of batch b-1 are
  interleaved into batch b's attention windows so the PE stays busy while
  Act grinds exp.
All matmuls bf16 with fp32 PSUM accumulation.
"""
import sys

sys.path.insert(0, "/opt/trn_rl_repo")

import numpy as np
import ml_dtypes

import concourse.bacc as bacc
import concourse.tile as tile
from concourse import mybir

B, S, D, H, DK = 4, 2048, 1024, 16, 64
NCORES = 8
JC = (H // NCORES) * DK  # 128 head-columns per core
BS = B * S  # 8192
SQC = 512  # q-window / projection free-dim chunk
NSQ = S // SQC  # 4 q-windows per batch
NSK = S // 128  # 16 key chunks per batch
NIC = D // 128  # 8 contraction chunks for projections
NSC = BS // SQC  # 16 s-chunks for projections
NSCB = NSC // B  # 4 s-chunks per batch
NICOUT = D // 128  # 8 output i-chunks

BF16 = mybir.dt.bfloat16
F32 = mybir.dt.float32
FP8 = mybir.dt.float8e4
AF = mybir.ActivationFunctionType
ALU = mybir.AluOpType
bf16 = ml_dtypes.bfloat16

USE_FP8_SCORES = True  # q/k cast to fp8e4, scores via DoubleRow (2x PE rate)
QK_SCALE = 4.0  # per-tensor scale folded into the fp8 cast; exp un-scales

_CACHE = {}


def _build_bass(niter=1):
    from contextlib import nullcontext

    nc = bacc.Bacc("TRN2", target_bir_lowering=False, debug=False)
    xq = nc.dram_tensor("xq", [NSC, 128, NIC, SQC], BF16, kind="ExternalInput")
    xk = nc.dram_tensor("xk", [NSC, 128, NIC, SQC], BF16, kind="ExternalInput")
    xv = nc.dram_tensor("xv", [NSC, 128, NIC, SQC], BF16, kind="ExternalInput")
    wq = nc.dram_tensor("wq", [128, NIC, JC], BF16, kind="ExternalInput")
    wk = nc.dram_tensor("wk", [128, NIC, JC], BF16, kind="ExternalInput")
    wv = nc.dram_tensor("wv", [128, NIC, JC], BF16, kind="ExternalInput")
    wo = nc.dram_tensor("wo", [JC, D], BF16, kind="ExternalInput")
    bq = nc.dram_tensor("bq", [JC, 1], F32, kind="ExternalInput")
    bk = nc.dram_tensor("bk", [JC, 1], F32, kind="ExternalInput")
    bv = nc.dram_tensor("bv", [1, JC], F32, kind="ExternalInput")
    ident = nc.dram_tensor("ident", [128, 128], BF16, kind="ExternalInput")
    outT = nc.dram_tensor("outT", [D, BS], BF16, kind="ExternalOutput")

    gp = nc.gpsimd  # Pool engine: elementwise epilogues / memsets / copies
    ve = nc.vector  # DVE: reciprocals + out-proj PSUM->SBUF casts

    with tile.TileContext(nc) as tc:
        with (
            tc.tile_pool(name="consts", bufs=1) as consts,
            tc.tile_pool(name="xin", bufs=4) as xin,
            tc.tile_pool(name="big", bufs=1) as big,
            tc.tile_pool(name="work", bufs=1) as work,
            tc.tile_pool(name="ps", bufs=1, space="PSUM") as ps,
            tc.For_i(0, niter, 1) if niter > 1 else nullcontext(),
        ):
            wq_sb = consts.tile([128, NIC, JC], BF16)
            wk_sb = consts.tile([128, NIC, JC], BF16)
            wv_sb = consts.tile([128, NIC, JC], BF16)
            wo_sb = consts.tile([JC, D], BF16)
            bq_sb = consts.tile([JC, 1], F32)
            bk_sb = consts.tile([JC, 1], F32)
            bvb_sb = consts.tile([128, JC], F32)
            id_sb = consts.tile([128, 128], BF16)
            warm = consts.tile([1, 2], F32)
            # batch-0 x chunks (big) lead; small weight/bias tiles interleave
            # so the first k/q projections start as early as possible
            xt_k0 = xin.tile([128, NIC, SQC], BF16, tag="xt")
            nc.sync.dma_start(xt_k0[:], xk[0])
            nc.sync.dma_start(wk_sb[:], wk[:])
            nc.sync.dma_start(bk_sb[:], bk[:])
            xt_q0 = xin.tile([128, NIC, SQC], BF16, tag="xt")
            nc.sync.dma_start(xt_q0[:], xq[0])
            nc.sync.dma_start(wq_sb[:], wq[:])
            nc.sync.dma_start(bq_sb[:], bq[:])
            xt_v0 = xin.tile([128, NIC, SQC], BF16, tag="xt")
            nc.sync.dma_start(xt_v0[:], xv[0])
            nc.sync.dma_start(wv_sb[:], wv[:])
            nc.sync.dma_start(id_sb[:], ident[:])
            # broadcast the v bias across all 128 partitions (free-step-0 source)
            nc.sync.dma_start(bvb_sb[:], bv[:, None, :].broadcast_to([1, 128, JC]))
            nc.sync.dma_start(wo_sb[:], wo[:])  # needed latest (out-projection)
            # front-load the Exp LUT while consts stream in
            gp.memset(warm[:], 0.0)
            nc.scalar.activation(warm[:, 0:1], warm[:, 1:2], AF.Exp, scale=1.0)

            if USE_FP8_SCORES:
                # head-dim split layout for DoubleRow: partition h*32+p holds
                # d = g*32+p at free slot g
                q8_sb = big.tile([64, 2, BS], FP8)
                k8_sb = big.tile([64, 2, BS], FP8)
            else:
                qT_sb = big.tile([128, BS], BF16)
                kT_sb = big.tile([128, BS], BF16)
            v1_sb = big.tile([128, B, 2, NSK, 65], BF16)
            gp.memset(v1_sb[:], 1.0)  # ones column at [..., 64]; rest overwritten

            # ---- emission helpers (instruction order == per-engine exec order) ----
            def proj_qk(xdram, w_sb, b_sb, dest, sc, xt=None):
                """One 512-col chunk of a transposed q/k projection + bias cast."""
                if xt is None:
                    xt = xin.tile([128, NIC, SQC], BF16, tag="xt")
                    nc.sync.dma_start(xt[:], xdram[sc])
                pq = ps.tile([128, SQC], F32, tag="mm", bufs=2)
                for ic in range(NIC):
                    nc.tensor.matmul(
                        pq[:], w_sb[:, ic, :], xt[:, ic, :],
                        start=(ic == 0), stop=(ic == NIC - 1),
                    )
                if USE_FP8_SCORES:
                    stag = work.tile([128, SQC], FP8, tag="stag", bufs=3)
                    ve.tensor_scalar(
                        stag[:], pq[:], b_sb[:, 0:1], QK_SCALE,
                        op0=ALU.add, op1=ALU.mult,
                    )
                    for g in range(2):
                        for h in range(2):
                            gp.dma_start(
                                dest[h * 32 : (h + 1) * 32, g,
                                     sc * SQC : (sc + 1) * SQC],
                                stag[h * 64 + g * 32 : h * 64 + g * 32 + 32, :],
                            )
                else:
                    ve.tensor_scalar(
                        dest[:, sc * SQC : (sc + 1) * SQC], pq[:],
                        b_sb[:, 0:1], None, op0=ALU.add,
                    )

            def proj_v(sc, xt=None):
                """One 512-row chunk of the natural-layout v projection."""
                if xt is None:
                    xt = xin.tile([128, NIC, SQC], BF16, tag="xt")
                    nc.sync.dma_start(xt[:], xv[sc])
                for sub in range(SQC // 128):
                    pv = ps.tile([128, SQC], F32, tag="mm", bufs=2)
                    for ic in range(NIC):
                        nc.tensor.matmul(
                            pv[:, 0:JC], xt[:, ic, sub * 128 : (sub + 1) * 128],
                            wv_sb[:, ic, :],
                            start=(ic == 0), stop=(ic == NIC - 1),
                        )
                    sg = sc * (SQC // 128) + sub
                    b_, skc = divmod(sg, NSK)
                    for h in range(2):
                        ve.tensor_tensor(
                            v1_sb[:, b_, h, skc, 0:64],
                            pv[:, h * 64 : (h + 1) * 64],
                            bvb_sb[:, h * 64 : (h + 1) * 64],
                            op=ALU.add,
                        )

            def outproj(b_, scb, ic):
                """One [128, 512] unit of the out-projection."""
                pf = ps.tile([128, SQC], F32, tag="mm", bufs=2)
                nc.tensor.matmul(
                    pf[:], wo_sb[:, ic * 128 : (ic + 1) * 128],
                    aoT_tiles[b_][:, scb, :, :],
                    start=True, stop=True,
                )
                ft = work.tile([128, SQC], BF16, tag="ft", bufs=4)
                ve.tensor_copy(ft[:], pf[:])
                gp.dma_start(
                    outT[ic * 128 : (ic + 1) * 128,
                         (b_ * NSCB + scb) * SQC : (b_ * NSCB + scb + 1) * SQC],
                    ft[:],
                )

            # deferred work queue, drained inside attention windows
            pending = []

            def drain(n):
                for _ in range(min(n, len(pending))):
                    pending.pop(0)()

            aoT_tiles = {}
            qdst = q8_sb if USE_FP8_SCORES else qT_sb
            kdst = k8_sb if USE_FP8_SCORES else kT_sb

            # ---- batch-0 projections (pipeline head) ----
            # just-in-time order: window 0 head 0 consumes k/v chunk c at
            # score-pair 2c, and only needs q chunk 0; q chunks 1-3
            # interleave into the first windows
            proj_qk(xk, wk_sb, bk_sb, kdst, 0, xt=xt_k0)
            proj_qk(xq, wq_sb, bq_sb, qdst, 0, xt=xt_q0)
            proj_v(0, xt=xt_v0)
            for sc in range(1, NSCB):
                proj_qk(xk, wk_sb, bk_sb, kdst, sc)
                proj_v(sc)
            for sc in range(1, NSCB):
                pending.append(lambda s=sc: proj_qk(xq, wq_sb, bq_sb, qdst, s))

            for b_ in range(B):
                # queue next batch's projections (k first: windows need all of k/v)
                if b_ + 1 < B:
                    for sc in range(NSCB):
                        sc_n = (b_ + 1) * NSCB + sc
                        pending.append(lambda s=sc_n: proj_qk(xk, wk_sb, bk_sb, kdst, s))
                    for sc in range(NSCB):
                        sc_n = (b_ + 1) * NSCB + sc
                        pending.append(lambda s=sc_n: proj_v(s))
                    for sc in range(NSCB):
                        sc_n = (b_ + 1) * NSCB + sc
                        pending.append(lambda s=sc_n: proj_qk(xq, wq_sb, bq_sb, qdst, s))

                aoT = big.tile([128, NSQ, 4, 128], BF16, tag="aoT", bufs=2)
                aoT_tiles[b_] = aoT
                for sqc in range(NSQ):
                    w = slice(b_ * S + sqc * SQC, b_ * S + (sqc + 1) * SQC)
                    # transposed attn outputs for this window, both heads (bf16 PSUM)
                    pT = ps.tile([128, 4, 128], BF16, tag="pT", bufs=1)
                    for h in range(2):
                        hs = slice(h * 64, (h + 1) * 64)
                        po = ps.tile([128, SQC], F32, tag="po", bufs=1)
                        hs32 = slice(h * 32, (h + 1) * 32)
                        for pair in range(NSK // 2):
                            sps = ps.tile([128, 2 * SQC], F32, tag="sps", bufs=2)
                            for i in range(2):
                                skc = pair * 2 + i
                                kk = slice(b_ * S + skc * 128, b_ * S + (skc + 1) * 128)
                                if USE_FP8_SCORES:
                                    nc.tensor.matmul(
                                        sps[:, i * SQC : (i + 1) * SQC],
                                        k8_sb[hs32, :, kk], q8_sb[hs32, :, w],
                                        start=True, stop=True,
                                        perf_mode=mybir.MatmulPerfMode.DoubleRow,
                                    )
                                else:
                                    nc.tensor.matmul(
                                        sps[:, i * SQC : (i + 1) * SQC],
                                        kT_sb[hs, kk], qT_sb[hs, w],
                                        start=True, stop=True,
                                    )
                            pt = work.tile([128, 2 * SQC], BF16, tag="pt", bufs=4)
                            nc.scalar.activation(
                                pt[:], sps[:], AF.Exp,
                                scale=0.125 / (QK_SCALE * QK_SCALE) if USE_FP8_SCORES else 0.125,
                            )
                            for i in range(2):
                                skc = pair * 2 + i
                                for qc in range(4):
                                    # first write marks the bank's zero region;
                                    # later first-touches of still-pending bytes
                                    # write fresh, then accumulate
                                    nc.tensor.matmul(
                                        po[:, qc * 65 : qc * 65 + 65],
                                        pt[:, i * SQC + qc * 128 : i * SQC + (qc + 1) * 128],
                                        v1_sb[:, b_, h, skc, :],
                                        start=(skc == 0 and qc == 0),
                                        stop=(skc == NSK - 1),
                                        skip_group_check=True,
                                    )
                        # normalize: per-q reciprocal of the ones-column sums
                        r4 = work.tile([128, 4], F32, tag="r4", bufs=2)
                        ve.reciprocal(r4[:], po[:, 64 : 4 * 65 : 65])
                        ao_n = work.tile([128, 4, 64], BF16, tag="ao_n", bufs=2)
                        for qc in range(4):
                            ve.tensor_scalar(
                                ao_n[:, qc, :], po[:, qc * 65 : qc * 65 + 64],
                                r4[:, qc : qc + 1], None, op0=ALU.mult,
                            )
                        for qc in range(4):
                            nc.tensor.matmul(
                                pT[hs, qc, :], ao_n[:, qc, :], id_sb[:],
                                is_transpose=True, start=(qc == 0), stop=True,
                                skip_group_check=True,
                                tile_position=(0, h * 64),
                            )
                        drain(1 if b_ == 0 else (6 if b_ == B - 1 else 4))
                    ve.tensor_copy(aoT[:, sqc, :, :], pT[:])
                    drain(1 if b_ == 0 else (4 if b_ == B - 1 else 3))
                    if b_ == B - 1:
                        # last batch: its out-projection windows self-interleave
                        for ic in range(NICOUT):
                            pending.append(lambda s=sqc, i=ic: outproj(B - 1, s, i))
                if b_ < B - 1:
                    # queue this batch's out-projection (drained during next batch)
                    for scb in range(NSCB):
                        for ic in range(NICOUT):
                            pending.append(lambda bb=b_, s=scb, i=ic: outproj(bb, s, i))
                if b_ == 0:
                    drain(4)
            # pipeline tail: whatever is left
            drain(len(pending))
    nc.finalize()
    return nc


def _chunk_xT(x):
    """[B,S,D] f32 -> xT chunked [NSC, 128, NIC, SQC] bf16 (shared by all cores)."""
    xT = np.ascontiguousarray(x.reshape(BS, D).T.astype(bf16))  # [D, BS]
    return np.ascontiguousarray(
        xT.reshape(NIC, 128, NSC, SQC).transpose(2, 1, 0, 3)
    )


def _prep_inputs(query, key, value, Wq, bq, Wk, bk, Wv, bv, Wo):
    xq = _chunk_xT(query)
    xk = _chunk_xT(key)
    xv = _chunk_xT(value)
    ident = np.eye(128, dtype=bf16)
    in_maps = []
    for c in range(NCORES):
        sl = slice(c * JC, (c + 1) * JC)

        def wT(W):  # [1024,128] -> [128, NIC, JC] chunked lhsT layout
            t = np.ascontiguousarray(W[sl, :].T.astype(bf16))  # [D, JC]
            return np.ascontiguousarray(t.reshape(NIC, 128, JC).transpose(1, 0, 2))

        in_maps.append(
            {
                "xq": xq,
                "xk": xk,
                "xv": xv,
                "wq": wT(Wq),
                "wk": wT(Wk),
                "wv": wT(Wv),
                "wo": np.ascontiguousarray(Wo[:, sl].T.astype(bf16)),  # [JC, D]
                "bq": np.asarray(bq[sl], np.float32).reshape(JC, 1),
                "bk": np.asarray(bk[sl], np.float32).reshape(JC, 1),
                "bv": np.asarray(bv[sl], np.float32).reshape(1, JC),
                "ident": ident,
            }
        )
    return in_maps


IN_NAMES = ["xq", "xk", "xv", "wq", "wk", "wv", "wo", "bq", "bk", "bv", "ident"]


def _get_mesh():
    import jax
    from jax.sharding import Mesh

    if "mesh" not in _CACHE:
        devices = jax.devices()[:NCORES]
        _CACHE["mesh"] = Mesh(np.asarray(devices), ("core",))
    return _CACHE["mesh"]


def _jitted_chain(niter):
    """Jitted runner for the Bass program with `niter` in-program iterations."""
    import jax
    from jax.sharding import PartitionSpec
    from jax.experimental.shard_map import shard_map
    from concourse import bass2jax

    key = ("jit", niter)
    if key in _CACHE:
        return _CACHE[key]

    nc = _CACHE.get(("nc", niter))
    if nc is None:
        nc = _CACHE[("nc", niter)] = _build_bass(niter)

    bass2jax.install_neuronx_cc_hook()
    out_avals = (jax.core.ShapedArray((D, BS), bf16),)
    part_name = nc.partition_id_tensor.name if nc.partition_id_tensor else None

    def _body(*args):
        operands = list(args)
        names = tuple(IN_NAMES)
        if part_name is not None:
            operands.append(bass2jax.partition_id_tensor())
            names = names + (part_name,)
        outs = bass2jax._bass_exec_p.bind(
            *operands,
            out_avals=out_avals,
            in_names=names,
            out_names=("outT",),
            lowering_input_output_aliases=(),
            sim_require_finite=True,
            sim_require_nnan=True,
            nc=nc,
        )
        return outs[0]

    fn = jax.jit(
        shard_map(
            _body,
            mesh=_get_mesh(),
            in_specs=(PartitionSpec("core"),) * len(IN_NAMES),
            out_specs=PartitionSpec("core"),
            check_rep=False,
        ),
        keep_unused=True,
    )
    _CACHE[key] = fn
    return fn


def _concat_inputs(in_maps):
    return [np.concatenate([m[name] for m in in_maps], axis=0) for name in IN_NAMES]


def _device_inputs(in_maps):
    """Stage per-core inputs onto the 8 devices once; reusable across calls."""
    import jax
    from jax.sharding import NamedSharding, PartitionSpec

    sh = NamedSharding(_get_mesh(), PartitionSpec("core"))
    return [jax.device_put(a, sh) for a in _concat_inputs(in_maps)]


def _timed_chain(in_maps, niter):
    """Wall-time one dispatch of the niter-iteration Bass program on
    device-resident inputs (the loop runs on-device; RPC cost is constant)."""
    import time

    dev = _CACHE.get("dev_inputs")
    if dev is None:
        dev = _CACHE["dev_inputs"] = _device_inputs(in_maps)
    fn = _jitted_chain(niter)
    fn(*dev).block_until_ready()  # compile+warm
    t0 = time.perf_counter()
    fn(*dev).block_until_ready()
    return time.perf_counter() - t0


def kernel(query, key, value, Wq, bq, Wk, bk, Wv, bv, Wo, bo):
    in_maps = _prep_inputs(query, key, value, Wq, bq, Wk, bk, Wv, bv, Wo)
    fn = _jitted_chain(1)
    out = np.asarray(fn(*_concat_inputs(in_maps)))  # [8*D, BS]
    acc = out[0:D].astype(np.float32)
    for c in range(1, NCORES):
        acc += out[c * D : (c + 1) * D]
    res = acc.T.reshape(B, S, D) + np.asarray(bo, np.float32)
    return np.ascontiguousarray(res.astype(np.float32))
